# revision 18
# baseline (speedup 1.0000x reference)
"""Exponential smoothing (per-channel EMA over time) on 8 Trainium2 cores.

  s_0 = x_0 ; s_t = a * x_t + (1 - a) * s_{t-1},  a = sigmoid(alpha)  (per channel)

Full shapes: x (16, 4096, 512) f32, alpha (1, 1, 512) f32 -> out (16, 4096, 512).
Sharding: data-parallel over batch B (16 -> 2 per core); alpha replicated.

Per core, per 2048-step time chunk:
  1. DMA-loads x (cast to bf16 on host) in native layout (t on partitions)
     via the Sync HWDGE queue; alpha rides the Scalar HWDGE queue in parallel
     so the x stream starts immediately after the program prologue.
  2. Transposes 128x128 blocks on the tensor engine into 2-bank PSUM tiles
     (time on the free axis, channels on partitions).
  3. Runs a hand-built custom DVE op (EMA_PAGED_ANT, registered below) that
     scans r_t = w*r_{t-1} + x_t directly out of PSUM at ~1.1 cyc/element,
     writing bf16 r to SBUF. The scan is the rescaled form r = s/a, so no
     pre-scale pass is needed; chunk chaining passes the previous chunk's
     last column as the carry. Chunk 0 seeds with r_{-1} = x_0/a, making
     s_0 = x_0 exactly. The first chunk is split (256+1792) so the vector
     engine starts as soon as the first 256KB of x lands; the last chunk is
     split in halves so the tail back-transposes overlap the final scans.
  4. Transposes back via a regular matmul against diag(a) (contracting the
     channel partitions), so s = a*r is applied by the tensor engine for free.
  5. Scalar engine evacuates PSUM -> bf16 SBUF; stores to HBM alternate
     between the GpSimd SWDGE queue and the Sync HWDGE queue so the output
     stream is never bound by a single ~210 GB/s queue.
Host converts the bf16 y back to f32.
"""

from contextlib import ExitStack

import ml_dtypes
import numpy as np

import concourse.bass as bass
import concourse.dve_ops as dve_ops
import concourse.tile as tile
from concourse import bacc, mybir
from concourse.bass_utils import run_bass_kernel_spmd
from concourse.dve_ops import DveOp
from concourse.dve_spec import (
    C0,
    C1,
    AluOp,
    Bin,
    One,
    Spec,
    Src0,
    Src1,
    _Placement,
    _Stage,
    _State,
    _assemble,
    COUNT_ONCE,
    PREV,
)
from concourse.dve_uop import AluInp, DveOpSpec, OutSel, Trigger
from concourse.masks import make_identity

# ---------------------------------------------------------------------------
# Custom DVE op: paged EMA scan, r_k = w*r_{k-1} + u_k at ~1.1 cyc/element.
# Within each 16-element page the weights are formed as w^(i+1) * w^-(j+1)
# via three no-bubble running products/sums; two non-consuming bubble uOps at
# each page boundary rescale the carry by w^16 exactly. fp32 range needs
# w^-15 finite: OK for w >= sigmoid(-5.8).
# ---------------------------------------------------------------------------

CURR = AluInp.CURR_ALU_OUT
SWAP = AluInp.CURR_SWAP_OUT
LANE_M = AluInp.PREV_DELAY_3
LANE_R = AluInp.PREV_DELAY_4
PAGE = 16
_BYP = _Stage(AluOp.BYPASS, PREV)


def _build_ema_uops():
    m_key = Bin(AluOp.MULTIPLY, Src0, C1)
    r_key = Bin(AluOp.MULTIPLY, C0, C0)
    p = _Placement(
        pipeline=[
            _Stage(AluOp.MULTIPLY, CURR, C1),     # st0: Inv <- Inv * (1/w)
            _Stage(AluOp.MULTIPLY, Src0, PREV),   # st1: m = u * Inv
            _Stage(AluOp.MULTIPLY, CURR, C0),     # st2: R <- R * w
            _Stage(AluOp.ADD, CURR, LANE_M),      # st3: A <- A + m
            _Stage(AluOp.MULTIPLY, PREV, LANE_R), # st4: out = A * R
            _BYP, _BYP, _BYP,
        ],
        node_stage={},
        lane={Src0: 0, C0: 1, C1: 2, m_key: 3, r_key: 4, One: 5},
        out_sel=OutSel.ALU_OUT,
        accum_stage=None,
        captures=[(2, 3), (3, 4)],
    )
    latch_p = _Placement(
        pipeline=[_BYP] * 8, node_stage={}, lane={Src1: 0},
        out_sel=OutSel.ALU_OUT, accum_stage=None, captures=[],
    )
    states = [
        _State(  # 0: latch-init — park the carry (in1) in st3's swap flop
            placement=latch_p, trigger=COUNT_ONCE, repeat=1,
            consume=(False, True),
            overrides={3: _Stage(AluOp.BYPASS, Src1, Src1, swap=True)},
            write_out=False, next=(1, 0, 0),
        ),
        _State(  # 1: seed — Inv=1, R=1, A=carry
            placement=p, trigger=COUNT_ONCE, repeat=1, consume=(False, False),
            overrides={
                0: _Stage(AluOp.BYPASS, One),
                2: _Stage(AluOp.BYPASS, One),
                3: _Stage(AluOp.BYPASS, SWAP),
            },
            write_out=False, next=(2, 0, 0),
        ),
        _State(  # 2: steady — 1 element/cycle; page wrap -> bubbles
            placement=p,
            trigger=(Trigger.SRC_TENSOR_DONE, Trigger.SUB_DIM_DONE, Trigger.NONE),
            next=(0, 3, 0), repeat=0, consume=(True, False), write_out=True,
        ),
        _State(  # 3: B1 — A <- A * w^PAGE (R held at st2)
            placement=p, trigger=COUNT_ONCE, repeat=1, consume=(False, False),
            overrides={
                0: _Stage(AluOp.BYPASS, CURR), 1: _BYP,
                2: _Stage(AluOp.BYPASS, CURR),
                3: _Stage(AluOp.MULTIPLY, CURR, PREV), 4: _BYP,
            },
            write_out=False, next=(4, 0, 0),
        ),
        _State(  # 4: B2 — reset Inv/R for the new page, hold A
            placement=p, trigger=COUNT_ONCE, repeat=1, consume=(False, False),
            overrides={
                0: _Stage(AluOp.BYPASS, One), 1: _BYP,
                2: _Stage(AluOp.BYPASS, One),
                3: _Stage(AluOp.BYPASS, CURR), 4: _BYP,
            },
            write_out=False, next=(2, 0, 0),
        ),
    ]
    uops = [_assemble(s) for s in states]
    for u in uops:
        u.validate("v3")
    return uops


def _ema_ref(in0, in1, s0, s1, imm2):
    P = in0.shape[0]
    u = in0.astype(np.float64).reshape(P, -1)
    w = np.asarray(s0, np.float64).reshape(P, 1)
    r = np.asarray(in1, np.float64).reshape(P, 1)[:, 0].copy()
    out = np.empty_like(u)
    for t in range(u.shape[1]):
        r = w[:, 0] * r + u[:, t]
        out[:, t] = r
    return out.reshape(in0.shape).astype(np.float32)


class _HandDveOp(DveOp):
    def compile(self, ver):
        if ver != "v3":
            raise ValueError(f"{self.name}: hand-built for v3/TRN2 only")
        cached = dve_ops._COMPILE_CACHE.get((self.name, ver))
        if cached is not None:
            return cached
        spec = DveOpSpec(
            name=self.name,
            opcode=dve_ops.get_dve_sub_opcode(self.name),
            uops=_build_ema_uops(),
            rd1_en=True,
        )
        dve_ops._COMPILE_CACHE[(self.name, ver)] = spec
        return spec


def _register_ema_op() -> DveOp:
    for op in dve_ops.OPS:
        if op.name == "EMA_PAGED_ANT":
            return op
    op = _HandDveOp(
        "EMA_PAGED_ANT",
        Spec(
            body=Bin(AluOp.ADD, Bin(AluOp.MULTIPLY, Src0, C1),
                     Bin(AluOp.MULTIPLY, Src1, C0)),
            reference=_ema_ref,
        ),
        subdim=True,
        uops_sha={},
    )
    dve_ops.OPS.append(op)
    dve_ops.CUSTOM_DVE_SPECS[op.name] = op.spec
    dve_ops._SUB_OPCODE_FOR_NAME[op.name] = (
        dve_ops._CUSTOM_DVE_ROW_BASE + len(dve_ops.OPS) - 1
    )
    assert dve_ops._SUB_OPCODE_FOR_NAME[op.name] < 0x20
    return op


# ---------------------------------------------------------------------------
# Kernel
# ---------------------------------------------------------------------------

B, T, D = 16, 4096, 512
NCORES = 8
BL = B // NCORES   # batches per core
P = 128            # partitions
TCW = 2048         # time chunk per pipeline iteration
ND = D // P        # channel chunks of 128 (4)
NK = TCW // P      # 128-row sub-chunks per time chunk (16)

FP32 = mybir.dt.float32
BF16 = mybir.dt.bfloat16


def build_program(bl: int = BL, t: int = T) -> bacc.Bacc:
    ema = _register_ema_op()
    nc = bacc.Bacc(
        "TRN2",
        target_bir_lowering=False,
        debug=False,
        enable_asserts=False,
        num_devices=NCORES,
    )
    x = nc.dram_tensor("x", (bl, t, D), BF16, kind="ExternalInput").ap()
    # Host-precomputed per-channel tables: (w | 1/w | 1/a | a), channel
    # d = j*128 + p laid out as [128, 4*ND].  Avoids the sigmoid
    # activation-table-load + reciprocal chain at startup.
    wt = nc.dram_tensor("wt", (P, 4 * ND), FP32, kind="ExternalInput").ap()
    y = nc.dram_tensor("y", (bl, t, D), BF16, kind="ExternalOutput").ap()

    with tile.TileContext(nc) as tc, ExitStack() as ctx:
        const_pool = ctx.enter_context(tc.tile_pool(name="const", bufs=1))
        xn_pool = ctx.enter_context(tc.tile_pool(name="xn", bufs=3))
        pin_pool = ctx.enter_context(tc.tile_pool(name="pin", bufs=2, space="PSUM"))
        pout_pool = ctx.enter_context(tc.tile_pool(name="pout", bufs=2, space="PSUM"))
        s_pool = ctx.enter_context(tc.tile_pool(name="s", bufs=20))
        y_pool = ctx.enter_context(tc.tile_pool(name="y", bufs=3))
        carry_pool = ctx.enter_context(tc.tile_pool(name="carry", bufs=1))

        ident = const_pool.tile([P, P], BF16)
        make_identity(nc, ident[:])

        # Per-channel tables ride the Scalar HWDGE queue so the Sync queue
        # starts streaming x immediately.
        wt_sb = const_pool.tile([P, 4 * ND], FP32)
        nc.scalar.dma_start(wt_sb[:], wt)
        w_sb = wt_sb[:, 0:ND]
        inv_w = wt_sb[:, ND : 2 * ND]
        inv_a = wt_sb[:, 2 * ND : 3 * ND]
        a_sb = wt_sb[:, 3 * ND : 4 * ND]

        # diag(a) per channel chunk: ident row p scaled by a[p] (bf16), on
        # the scalar engine (cheap per-partition scale).
        diags = []
        for j in range(ND):
            dg = const_pool.tile([P, P], BF16, tag=f"diag{j}")
            nc.scalar.mul(dg[:], ident[:], a_sb[:, j : j + 1])
            diags.append(dg)

        inits = carry_pool.tile([P, bl * ND], FP32)

        def back_chunk(ss, b, t0, m0, m1, yout, evac_engines=(nc.scalar,)):
            """Back-transpose (diag(a) matmul) + PSUM evac for k-pairs
            m0..m1 of the chunk whose scan outputs are `ss`."""
            for m in range(m0, m1):
                pout = pout_pool.tile([P, 2 * D], FP32, tag="pout")
                for h in range(2):
                    k = 2 * m + h
                    for j in range(ND):
                        nc.tensor.matmul(
                            pout[:, h * D + j * P : (h * D + (j + 1) * P)],
                            ss[j][:, k * P : (k + 1) * P],
                            diags[j][:],
                        )
                eng = evac_engines[m % len(evac_engines)]
                if eng is nc.vector:
                    nc.vector.tensor_copy(yout[:, 2 * m : 2 * m + 2, :], pout[:])
                else:
                    eng.copy(yout[:, 2 * m : 2 * m + 2, :], pout[:])

        def out_dma(eng, b, t0, r0, r1, yout):
            eng.dma_start(
                y[b, t0 + r0 : t0 + r1, :].rearrange("(k p) d -> p k d", p=P),
                yout[:, r0 // P : r1 // P, :],
            )

        # Chunk schedule: (b, t0, clen).  Geometric ramp: the early chunks
        # are small so the vector engine starts on the first 256KB of x and
        # keeps pace with the input-DMA ramp; the tensor engine then stays
        # continuously busy, which also warms the PE HAM clock-gate early.
        chunks = [
            (0, 0, 256),
            (1, 0, 256),
            (0, 256, 768),
            (1, 256, 768),
            (0, 1024, 1024),
            (1, 1024, 1024),
            (0, 2048, 2048),
            (1, 2048, 2048),
        ]
        s_prevs = [None] * bl
        pending = None  # deferred back-pass: (ss, b, t0, clen)
        for ci, (b, t0, clen) in enumerate(chunks):
            last = ci == len(chunks) - 1
            nk = clen // P
            xn = xn_pool.tile([P, NK, D], BF16, tag="xn")
            # Input pieces (Sync HWDGE, input-only so never blocked):
            if clen <= 768:
                pieces = (0, clen)
            else:
                pieces = (0, clen // 2, clen)
            for r0, r1 in zip(pieces[:-1], pieces[1:]):
                nc.sync.dma_start(
                    xn[:, r0 // P : r1 // P, :],
                    x[b, t0 + r0 : t0 + r1, :].rearrange("(k p) d -> p k d", p=P),
                )

            ss = []
            scan_args = []
            for j in range(ND):
                pin = pin_pool.tile([P, TCW], BF16, tag="pin")
                for k in range(nk):
                    nc.tensor.transpose(
                        pin[:, k * P : (k + 1) * P],
                        xn[:, k, j * P : (j + 1) * P],
                        ident[:],
                    )
                if t0 == 0:
                    # carry r_{-1} = x_0 / a  =>  s_0 = x_0 exactly
                    nc.scalar.mul(
                        inits[:, b * ND + j : b * ND + j + 1],
                        pin[:, 0:1],
                        inv_a[:, j : j + 1],
                    )
                    carry = inits[:, b * ND + j : b * ND + j + 1]
                else:
                    prev_s, prev_len = s_prevs[b]
                    carry = prev_s[j][:, prev_len - 1 : prev_len]
                s = s_pool.tile([P, TCW], BF16, tag="s", name=f"s_{b}_{t0}_{j}")
                ss.append(s)
                scan_args.append((pin, carry, s))

            def emit_scans(c0, c1):
                for j, (pin, carry, s) in enumerate(scan_args):
                    cr = carry if c0 == 0 else s[:, c0 - 1 : c0]
                    nc.vector._custom_dve(
                        ema,
                        out=s[:, c0:c1].rearrange("p (s n) -> p s n", n=PAGE),
                        in0=pin[:, c0:c1].rearrange("p (s n) -> p s n", n=PAGE),
                        in1=cr,
                        s0=w_sb[:, j : j + 1],
                        s1=inv_w[:, j : j + 1],
                    )

            if not last:
                emit_scans(0, clen)
                s_prevs[b] = (ss, clen)
                # Deferred back-pass of the PREVIOUS chunk: its scans are done
                # by now, so the in-order tensor queue never stalls ahead of
                # vector-critical transposes.  Each output half is stored as
                # soon as its evacs land.
                if pending is not None:
                    pss, pb, pt0, pclen = pending
                    pnk = pclen // P
                    yout = y_pool.tile([P, NK, D], BF16, tag="y")
                    if pclen <= 512:
                        back_chunk(pss, pb, pt0, 0, pnk // 2, yout)
                        out_dma(nc.gpsimd if ci % 2 else nc.scalar, pb, pt0, 0, pclen, yout)
                    else:
                        mh = max(1, pnk // 4)
                        back_chunk(pss, pb, pt0, 0, mh, yout)
                        out_dma(nc.gpsimd, pb, pt0, 0, mh * 2 * P, yout)
                        back_chunk(pss, pb, pt0, mh, pnk // 2, yout)
                        out_dma(nc.scalar, pb, pt0, mh * 2 * P, pclen, yout)
                pending = (ss, b, t0, clen)
            else:
                # Last chunk: scans in 512-step quarters with the back-pass
                # chasing each quarter; the final quarter's evacs run on the
                # (by then idle) vector engine and the last store rides the
                # (by then idle) Sync HWDGE queue, so the tail drains fast.
                Q = 512
                pss, pb, pt0, pclen = pending
                pnk = pclen // P
                pyout = y_pool.tile([P, NK, D], BF16, tag="y")
                yout = y_pool.tile([P, NK, D], BF16, tag="y")
                for q in range(4):
                    emit_scans(q * Q, (q + 1) * Q)
                    if q == 0:
                        # Previous chunk's deferred back-pass, stored in
                        # halves as the evacs land.
                        back_chunk(pss, pb, pt0, 0, pnk // 4, pyout)
                        out_dma(nc.gpsimd, pb, pt0, 0, pclen // 2, pyout)
                        back_chunk(pss, pb, pt0, pnk // 4, pnk // 2, pyout)
                        out_dma(nc.scalar, pb, pt0, pclen // 2, pclen, pyout)
                    else:
                        m0, m1 = (q - 1) * 2, q * 2
                        back_chunk(ss, b, t0, m0, m1, yout)
                        if q == 2:
                            out_dma(nc.gpsimd, b, t0, 0, clen // 2, yout)
                out_dma(nc.scalar, b, t0, clen // 2, clen * 3 // 4, yout)
                back_chunk(ss, b, t0, 6, 8, yout,
                           evac_engines=(nc.vector,))
                out_dma(nc.sync, b, t0, clen * 3 // 4, clen * 7 // 8, yout)
                out_dma(nc.scalar, b, t0, clen * 7 // 8, clen, yout)

    nc.compile()
    return nc


_prog = None


def make_in_maps(x, alpha):
    x = np.asarray(x)
    alpha = np.asarray(alpha, dtype=np.float64)
    assert x.shape == (B, T, D) and alpha.shape == (1, 1, D)
    xb = np.ascontiguousarray(x.astype(ml_dtypes.bfloat16))
    # Per-channel tables (w | 1/w | 1/a | a) as [128, 4*ND], channel
    # d = j*128 + p (so column j holds channels j*128..j*128+127).
    a = 1.0 / (1.0 + np.exp(-alpha[0, 0, :]))  # (D,)
    w = 1.0 - a
    wt = np.empty((P, 4 * ND), np.float32)
    for bi, tb in enumerate((w, 1.0 / w, 1.0 / a, a)):
        wt[:, bi * ND : (bi + 1) * ND] = tb.reshape(ND, P).T
    return [
        {"x": np.ascontiguousarray(xb[i * BL : (i + 1) * BL]), "wt": wt}
        for i in range(NCORES)
    ]


def kernel(x, alpha):
    global _prog
    if _prog is None:
        _prog = build_program()
    in_maps = make_in_maps(x, alpha)
    res = run_bass_kernel_spmd(_prog, in_maps, core_ids=list(range(NCORES)))
    out = np.concatenate([r["y"] for r in res.results], axis=0)
    return np.ascontiguousarray(out.astype(np.float32))


# revision 19
# speedup vs baseline: 1.1020x; 1.1020x over previous
"""Exponential smoothing (per-channel EMA over time) on 8 Trainium2 cores.

  s_0 = x_0 ; s_t = a * x_t + (1 - a) * s_{t-1},  a = sigmoid(alpha)  (per channel)

Full shapes: x (16, 4096, 512) f32, alpha (1, 1, 512) f32 -> out (16, 4096, 512).
Sharding: data-parallel over batch B (16 -> 2 per core); alpha replicated.

Per core, per 1024-step time chunk:
  1. DMA-loads x (cast to bf16 on host) in native layout (t on partitions)
     on the Sync HWDGE queue; the host-precomputed per-channel tables
     (w | 1/w | 1/a | a, padded to 512B rows for line-rate DMA) ride the
     Scalar HWDGE queue in parallel, so no sigmoid/reciprocal/activation-
     table chain gates the first scan.
  2. Transposes 128x128 blocks on the tensor engine into 1-bank PSUM tiles
     (time on the free axis, channels on partitions).  The k=0 transpose is
     emitted first and immediately feeds the carry-seed mul, so the first
     scan only waits on the first 256KB of x.
  3. Runs a hand-built custom DVE op (EMA_PAGED_ANT, registered below) that
     scans r_t = w*r_{t-1} + x_t directly out of PSUM at ~1.1 cyc/element
     (vs ~2.3 for the stock TensorTensorScanArith), writing bf16 r to SBUF.
     The scan is the rescaled form r = s/a, so no pre-scale pass is needed;
     chunk chaining passes the previous chunk's last column as the carry.
     Chunk 0 seeds with r_{-1} = x_0/a, making s_0 = x_0 exactly.
  4. Transposes back via a regular matmul against diag(a) (contracting the
     channel partitions), so s = a*r is applied by the tensor engine for free.
  5. Scalar engine evacuates PSUM -> bf16 SBUF; stores go out on the GpSimd
     SWDGE queue, except the final tile's, which use the (by then idle)
     Sync/Scalar HWDGE queues and the vector engine for the last evacs so
     the tail drains fast.
Host converts the bf16 y back to f32.
"""

from contextlib import ExitStack

import ml_dtypes
import numpy as np

import concourse.bass as bass
import concourse.dve_ops as dve_ops
import concourse.tile as tile
from concourse import bacc, mybir
from concourse.bass_utils import run_bass_kernel_spmd
from concourse.dve_ops import DveOp
from concourse.dve_spec import (
    C0,
    C1,
    AluOp,
    Bin,
    One,
    Spec,
    Src0,
    Src1,
    _Placement,
    _Stage,
    _State,
    _assemble,
    COUNT_ONCE,
    PREV,
)
from concourse.dve_uop import AluInp, DveOpSpec, OutSel, Trigger
from concourse.masks import make_identity

# ---------------------------------------------------------------------------
# Custom DVE op: paged EMA scan, r_k = w*r_{k-1} + u_k at ~1.1 cyc/element.
# Within each 16-element page the weights are formed as w^(i+1) * w^-(j+1)
# via three no-bubble running products/sums; two non-consuming bubble uOps at
# each page boundary rescale the carry by w^16 exactly. fp32 range needs
# w^-15 finite: OK for w >= sigmoid(-5.8).
# ---------------------------------------------------------------------------

CURR = AluInp.CURR_ALU_OUT
SWAP = AluInp.CURR_SWAP_OUT
LANE_M = AluInp.PREV_DELAY_3
LANE_R = AluInp.PREV_DELAY_4
PAGE = 16
_BYP = _Stage(AluOp.BYPASS, PREV)


def _build_ema_uops():
    m_key = Bin(AluOp.MULTIPLY, Src0, C1)
    r_key = Bin(AluOp.MULTIPLY, C0, C0)
    p = _Placement(
        pipeline=[
            _Stage(AluOp.MULTIPLY, CURR, C1),     # st0: Inv <- Inv * (1/w)
            _Stage(AluOp.MULTIPLY, Src0, PREV),   # st1: m = u * Inv
            _Stage(AluOp.MULTIPLY, CURR, C0),     # st2: R <- R * w
            _Stage(AluOp.ADD, CURR, LANE_M),      # st3: A <- A + m
            _Stage(AluOp.MULTIPLY, PREV, LANE_R), # st4: out = A * R
            _BYP, _BYP, _BYP,
        ],
        node_stage={},
        lane={Src0: 0, C0: 1, C1: 2, m_key: 3, r_key: 4, One: 5},
        out_sel=OutSel.ALU_OUT,
        accum_stage=None,
        captures=[(2, 3), (3, 4)],
    )
    latch_p = _Placement(
        pipeline=[_BYP] * 8, node_stage={}, lane={Src1: 0},
        out_sel=OutSel.ALU_OUT, accum_stage=None, captures=[],
    )
    states = [
        _State(  # 0: latch-init — park the carry (in1) in st3's swap flop
            placement=latch_p, trigger=COUNT_ONCE, repeat=1,
            consume=(False, True),
            overrides={3: _Stage(AluOp.BYPASS, Src1, Src1, swap=True)},
            write_out=False, next=(1, 0, 0),
        ),
        _State(  # 1: seed — Inv=1, R=1, A=carry
            placement=p, trigger=COUNT_ONCE, repeat=1, consume=(False, False),
            overrides={
                0: _Stage(AluOp.BYPASS, One),
                2: _Stage(AluOp.BYPASS, One),
                3: _Stage(AluOp.BYPASS, SWAP),
            },
            write_out=False, next=(2, 0, 0),
        ),
        _State(  # 2: steady — 1 element/cycle; page wrap -> bubbles
            placement=p,
            trigger=(Trigger.SRC_TENSOR_DONE, Trigger.SUB_DIM_DONE, Trigger.NONE),
            next=(0, 3, 0), repeat=0, consume=(True, False), write_out=True,
        ),
        _State(  # 3: B1 — A <- A * w^PAGE (R held at st2)
            placement=p, trigger=COUNT_ONCE, repeat=1, consume=(False, False),
            overrides={
                0: _Stage(AluOp.BYPASS, CURR), 1: _BYP,
                2: _Stage(AluOp.BYPASS, CURR),
                3: _Stage(AluOp.MULTIPLY, CURR, PREV), 4: _BYP,
            },
            write_out=False, next=(4, 0, 0),
        ),
        _State(  # 4: B2 — reset Inv/R for the new page, hold A
            placement=p, trigger=COUNT_ONCE, repeat=1, consume=(False, False),
            overrides={
                0: _Stage(AluOp.BYPASS, One), 1: _BYP,
                2: _Stage(AluOp.BYPASS, One),
                3: _Stage(AluOp.BYPASS, CURR), 4: _BYP,
            },
            write_out=False, next=(2, 0, 0),
        ),
    ]
    uops = [_assemble(s) for s in states]
    for u in uops:
        u.validate("v3")
    return uops


def _ema_ref(in0, in1, s0, s1, imm2):
    P = in0.shape[0]
    u = in0.astype(np.float64).reshape(P, -1)
    w = np.asarray(s0, np.float64).reshape(P, 1)
    r = np.asarray(in1, np.float64).reshape(P, 1)[:, 0].copy()
    out = np.empty_like(u)
    for t in range(u.shape[1]):
        r = w[:, 0] * r + u[:, t]
        out[:, t] = r
    return out.reshape(in0.shape).astype(np.float32)


class _HandDveOp(DveOp):
    def compile(self, ver):
        if ver != "v3":
            raise ValueError(f"{self.name}: hand-built for v3/TRN2 only")
        cached = dve_ops._COMPILE_CACHE.get((self.name, ver))
        if cached is not None:
            return cached
        spec = DveOpSpec(
            name=self.name,
            opcode=dve_ops.get_dve_sub_opcode(self.name),
            uops=_build_ema_uops(),
            rd1_en=True,
        )
        dve_ops._COMPILE_CACHE[(self.name, ver)] = spec
        return spec


def _register_ema_op() -> DveOp:
    for op in dve_ops.OPS:
        if op.name == "EMA_PAGED_ANT":
            return op
    op = _HandDveOp(
        "EMA_PAGED_ANT",
        Spec(
            body=Bin(AluOp.ADD, Bin(AluOp.MULTIPLY, Src0, C1),
                     Bin(AluOp.MULTIPLY, Src1, C0)),
            reference=_ema_ref,
        ),
        subdim=True,
        uops_sha={},
    )
    dve_ops.OPS.append(op)
    dve_ops.CUSTOM_DVE_SPECS[op.name] = op.spec
    dve_ops._SUB_OPCODE_FOR_NAME[op.name] = (
        dve_ops._CUSTOM_DVE_ROW_BASE + len(dve_ops.OPS) - 1
    )
    assert dve_ops._SUB_OPCODE_FOR_NAME[op.name] < 0x20
    return op


# ---------------------------------------------------------------------------
# Kernel
# ---------------------------------------------------------------------------

B, T, D = 16, 4096, 512
NCORES = 8
BL = B // NCORES   # batches per core
P = 128            # partitions
TCW = 1024         # time chunk per pipeline iteration
ND = D // P        # channel chunks of 128 (4)
NK = TCW // P      # 128-row sub-chunks per time chunk (8)
WTC = 128          # wt table padded to 128 f32 columns (512B rows)

FP32 = mybir.dt.float32
BF16 = mybir.dt.bfloat16


def build_program(bl: int = BL, t: int = T) -> bacc.Bacc:
    ema = _register_ema_op()
    ntc = t // TCW
    nc = bacc.Bacc(
        "TRN2",
        target_bir_lowering=False,
        debug=False,
        enable_asserts=False,
        num_devices=NCORES,
    )
    x = nc.dram_tensor("x", (bl, t, D), BF16, kind="ExternalInput").ap()
    wt = nc.dram_tensor("wt", (P, WTC), FP32, kind="ExternalInput").ap()
    y = nc.dram_tensor("y", (bl, t, D), BF16, kind="ExternalOutput").ap()

    with tile.TileContext(nc) as tc, ExitStack() as ctx:
        const_pool = ctx.enter_context(tc.tile_pool(name="const", bufs=1))
        xn_pool = ctx.enter_context(tc.tile_pool(name="xn", bufs=4))
        pin_pool = ctx.enter_context(tc.tile_pool(name="pin", bufs=4, space="PSUM"))
        pout_pool = ctx.enter_context(tc.tile_pool(name="pout", bufs=2, space="PSUM"))
        s_pool = ctx.enter_context(tc.tile_pool(name="s", bufs=10))
        y_pool = ctx.enter_context(tc.tile_pool(name="y", bufs=3))
        carry_pool = ctx.enter_context(tc.tile_pool(name="carry", bufs=1))

        ident = const_pool.tile([P, P], BF16)
        make_identity(nc, ident[:])

        # Host-precomputed per-channel tables (w | 1/w | 1/a | a), channel
        # d = j*128 + p, on the Scalar HWDGE queue (Sync streams x).
        wt_sb = const_pool.tile([P, WTC], FP32)
        nc.scalar.dma_start(wt_sb[:], wt)
        w_sb = wt_sb[:, 0:ND]
        inv_w = wt_sb[:, ND : 2 * ND]
        inv_a = wt_sb[:, 2 * ND : 3 * ND]
        a_sb = wt_sb[:, 3 * ND : 4 * ND]

        # diag(a) per channel chunk: ident row p scaled by a[p] (bf16).
        diags = []
        for j in range(ND):
            dg = const_pool.tile([P, P], BF16, tag=f"diag{j}")
            nc.scalar.mul(dg[:], ident[:], a_sb[:, j : j + 1])
            diags.append(dg)

        inits = carry_pool.tile([P, bl * ND], FP32)

        s_prevs = [[None] * ND for _ in range(bl)]
        for tci in range(ntc):
            for b in range(bl):
                t0 = tci * TCW
                xn = xn_pool.tile([P, NK, D], BF16, tag="xn")
                nsplit = 4 if (tci == 0 and b == 0) else 2
                kq, tq = NK // nsplit, TCW // nsplit
                for hh in range(nsplit):
                    nc.sync.dma_start(
                        xn[:, hh * kq : (hh + 1) * kq, :],
                        x[b, t0 + hh * tq : t0 + (hh + 1) * tq, :].rearrange(
                            "(k p) d -> p k d", p=P
                        ),
                    )

                # Transpose to (d-part, t-free) PSUM, then scan straight out
                # of PSUM with the custom EMA op.  The k=0 transpose goes
                # first so the carry-seed mul only waits on the first input
                # piece.
                ss = []
                for j in range(ND):
                    pin = pin_pool.tile([P, TCW], BF16, tag="pin")
                    nc.tensor.transpose(
                        pin[:, 0:P], xn[:, 0, j * P : (j + 1) * P], ident[:]
                    )
                    if tci == 0:
                        # carry r_{-1} = x_0 / a  =>  s_0 = x_0 exactly
                        nc.scalar.mul(
                            inits[:, b * ND + j : b * ND + j + 1],
                            pin[:, 0:1],
                            inv_a[:, j : j + 1],
                        )
                        carry = inits[:, b * ND + j : b * ND + j + 1]
                    else:
                        carry = s_prevs[b][j][:, TCW - 1 : TCW]
                    for k in range(1, NK):
                        nc.tensor.transpose(
                            pin[:, k * P : (k + 1) * P],
                            xn[:, k, j * P : (j + 1) * P],
                            ident[:],
                        )
                    s = s_pool.tile([P, TCW], BF16, tag="s", name=f"s{j}_{b}_{tci}")
                    nc.vector._custom_dve(
                        ema,
                        out=s[:].rearrange("p (s n) -> p s n", n=PAGE),
                        in0=pin[:].rearrange("p (s n) -> p s n", n=PAGE),
                        in1=carry,
                        s0=w_sb[:, j : j + 1],
                        s1=inv_w[:, j : j + 1],
                    )
                    ss.append(s)
                s_prevs[b] = ss

                # Transpose back with diag(a): yout = (a * r)^T = s^T.
                last = tci == ntc - 1 and b == bl - 1
                yout = y_pool.tile([P, NK, D], BF16, tag="y")
                for m in range(NK // 2):
                    pout = pout_pool.tile([P, 2 * D], FP32, tag="pout")
                    for h in range(2):
                        k = 2 * m + h
                        for j in range(ND):
                            nc.tensor.matmul(
                                pout[:, h * D + j * P : (h * D + (j + 1) * P)],
                                ss[j][:, k * P : (k + 1) * P],
                                diags[j][:],
                            )
                    if last and m % 2 == 1:
                        nc.vector.tensor_copy(yout[:, 2 * m : 2 * m + 2, :], pout[:])
                    else:
                        nc.scalar.copy(yout[:, 2 * m : 2 * m + 2, :], pout[:])
                half = TCW // 2
                for hh in range(2):
                    # Final tile's stores ride the idle Sync/Scalar HWDGE
                    # queues (fast completion); earlier tiles use GpSimd
                    # SWDGE so the compute-side queues stay free.
                    if last:
                        eng = nc.sync if hh == 0 else nc.scalar
                    else:
                        eng = nc.gpsimd
                    eng.dma_start(
                        y[b, t0 + hh * half : t0 + (hh + 1) * half, :].rearrange(
                            "(k p) d -> p k d", p=P
                        ),
                        yout[:, hh * (NK // 2) : (hh + 1) * (NK // 2), :],
                    )

    nc.compile()
    return nc


_prog = None


def make_in_maps(x, alpha):
    x = np.asarray(x)
    alpha = np.asarray(alpha, dtype=np.float64)
    assert x.shape == (B, T, D) and alpha.shape == (1, 1, D)
    xb = np.ascontiguousarray(x.astype(ml_dtypes.bfloat16))
    # Per-channel tables (w | 1/w | 1/a | a) as [128, WTC], channel
    # d = j*128 + p; padded to 512B rows so the DMA runs at line rate.
    a = 1.0 / (1.0 + np.exp(-alpha[0, 0, :]))  # (D,)
    w = 1.0 - a
    wt = np.zeros((P, WTC), np.float32)
    for bi, tb in enumerate((w, 1.0 / w, 1.0 / a, a)):
        wt[:, bi * ND : (bi + 1) * ND] = tb.reshape(ND, P).T
    return [
        {"x": np.ascontiguousarray(xb[i * BL : (i + 1) * BL]), "wt": wt}
        for i in range(NCORES)
    ]


def kernel(x, alpha):
    global _prog
    if _prog is None:
        _prog = build_program()
    in_maps = make_in_maps(x, alpha)
    res = run_bass_kernel_spmd(_prog, in_maps, core_ids=list(range(NCORES)))
    out = np.concatenate([r["y"] for r in res.results], axis=0)
    return np.ascontiguousarray(out.astype(np.float32))


# revision 21
# speedup vs baseline: 1.1127x; 1.0097x over previous
"""Exponential smoothing (per-channel EMA over time) on 8 Trainium2 cores.

  s_0 = x_0 ; s_t = a * x_t + (1 - a) * s_{t-1},  a = sigmoid(alpha)  (per channel)

Full shapes: x (16, 4096, 512) f32, alpha (1, 1, 512) f32 -> out (16, 4096, 512).
Sharding: data-parallel over batch B (16 -> 2 per core); alpha replicated.

Per core, per 1024-step time chunk:
  1. DMA-loads x (cast to bf16 on host) in native layout (t on partitions)
     on the Sync HWDGE queue; the host-precomputed per-channel tables
     (w | 1/w | 1/a | a, padded to 512B rows for line-rate DMA) ride the
     Scalar HWDGE queue in parallel, so no sigmoid/reciprocal/activation-
     table chain gates the first scan.
  2. Transposes 128x128 blocks on the tensor engine into 1-bank PSUM tiles
     (time on the free axis, channels on partitions).  The k=0 transpose is
     emitted first and immediately feeds the carry-seed mul, so the first
     scan only waits on the first 256KB of x.
  3. Runs a hand-built custom DVE op (EMA_PAGED_ANT, registered below) that
     scans r_t = w*r_{t-1} + x_t directly out of PSUM at ~1.1 cyc/element
     (vs ~2.3 for the stock TensorTensorScanArith), writing bf16 r to SBUF.
     The scan is the rescaled form r = s/a, so no pre-scale pass is needed;
     chunk chaining passes the previous chunk's last column as the carry.
     Chunk 0 seeds with r_{-1} = x_0/a, making s_0 = x_0 exactly.
  4. Transposes back via a regular matmul against diag(a) (contracting the
     channel partitions), so s = a*r is applied by the tensor engine for free.
  5. Scalar engine evacuates PSUM -> bf16 SBUF; stores go out on the GpSimd
     SWDGE queue, except the final tile's, which use the (by then idle)
     Sync/Scalar HWDGE queues and the vector engine for the last evacs so
     the tail drains fast.
Host converts the bf16 y back to f32.
"""

from contextlib import ExitStack

import ml_dtypes
import numpy as np

import concourse.bass as bass
import concourse.dve_ops as dve_ops
import concourse.tile as tile
from concourse import bacc, mybir
from concourse.bass_utils import run_bass_kernel_spmd
from concourse.dve_ops import DveOp
from concourse.dve_spec import (
    C0,
    C1,
    AluOp,
    Bin,
    One,
    Spec,
    Src0,
    Src1,
    _Placement,
    _Stage,
    _State,
    _assemble,
    COUNT_ONCE,
    PREV,
)
from concourse.dve_uop import AluInp, DveOpSpec, OutSel, Trigger
from concourse.masks import make_identity

# ---------------------------------------------------------------------------
# Custom DVE op: paged EMA scan, r_k = w*r_{k-1} + u_k at ~1.1 cyc/element.
# Within each 16-element page the weights are formed as w^(i+1) * w^-(j+1)
# via three no-bubble running products/sums; two non-consuming bubble uOps at
# each page boundary rescale the carry by w^16 exactly. fp32 range needs
# w^-15 finite: OK for w >= sigmoid(-5.8).
# ---------------------------------------------------------------------------

CURR = AluInp.CURR_ALU_OUT
SWAP = AluInp.CURR_SWAP_OUT
LANE_M = AluInp.PREV_DELAY_3
LANE_R = AluInp.PREV_DELAY_4
PAGE = 16
_BYP = _Stage(AluOp.BYPASS, PREV)


def _build_ema_uops():
    m_key = Bin(AluOp.MULTIPLY, Src0, C1)
    r_key = Bin(AluOp.MULTIPLY, C0, C0)
    p = _Placement(
        pipeline=[
            _Stage(AluOp.MULTIPLY, CURR, C1),     # st0: Inv <- Inv * (1/w)
            _Stage(AluOp.MULTIPLY, Src0, PREV),   # st1: m = u * Inv
            _Stage(AluOp.MULTIPLY, CURR, C0),     # st2: R <- R * w
            _Stage(AluOp.ADD, CURR, LANE_M),      # st3: A <- A + m
            _Stage(AluOp.MULTIPLY, PREV, LANE_R), # st4: out = A * R
            _BYP, _BYP, _BYP,
        ],
        node_stage={},
        lane={Src0: 0, C0: 1, C1: 2, m_key: 3, r_key: 4, One: 5},
        out_sel=OutSel.ALU_OUT,
        accum_stage=None,
        captures=[(2, 3), (3, 4)],
    )
    latch_p = _Placement(
        pipeline=[_BYP] * 8, node_stage={}, lane={Src1: 0},
        out_sel=OutSel.ALU_OUT, accum_stage=None, captures=[],
    )
    states = [
        _State(  # 0: latch-init — park the carry (in1) in st3's swap flop
            placement=latch_p, trigger=COUNT_ONCE, repeat=1,
            consume=(False, True),
            overrides={3: _Stage(AluOp.BYPASS, Src1, Src1, swap=True)},
            write_out=False, next=(1, 0, 0),
        ),
        _State(  # 1: seed — Inv=1, R=1, A=carry
            placement=p, trigger=COUNT_ONCE, repeat=1, consume=(False, False),
            overrides={
                0: _Stage(AluOp.BYPASS, One),
                2: _Stage(AluOp.BYPASS, One),
                3: _Stage(AluOp.BYPASS, SWAP),
            },
            write_out=False, next=(2, 0, 0),
        ),
        _State(  # 2: steady — 1 element/cycle; page wrap -> bubbles
            placement=p,
            trigger=(Trigger.SRC_TENSOR_DONE, Trigger.SUB_DIM_DONE, Trigger.NONE),
            next=(0, 3, 0), repeat=0, consume=(True, False), write_out=True,
        ),
        _State(  # 3: B1 — A <- A * w^PAGE (R held at st2)
            placement=p, trigger=COUNT_ONCE, repeat=1, consume=(False, False),
            overrides={
                0: _Stage(AluOp.BYPASS, CURR), 1: _BYP,
                2: _Stage(AluOp.BYPASS, CURR),
                3: _Stage(AluOp.MULTIPLY, CURR, PREV), 4: _BYP,
            },
            write_out=False, next=(4, 0, 0),
        ),
        _State(  # 4: B2 — reset Inv/R for the new page, hold A
            placement=p, trigger=COUNT_ONCE, repeat=1, consume=(False, False),
            overrides={
                0: _Stage(AluOp.BYPASS, One), 1: _BYP,
                2: _Stage(AluOp.BYPASS, One),
                3: _Stage(AluOp.BYPASS, CURR), 4: _BYP,
            },
            write_out=False, next=(2, 0, 0),
        ),
    ]
    uops = [_assemble(s) for s in states]
    for u in uops:
        u.validate("v3")
    return uops


def _ema_ref(in0, in1, s0, s1, imm2):
    P = in0.shape[0]
    u = in0.astype(np.float64).reshape(P, -1)
    w = np.asarray(s0, np.float64).reshape(P, 1)
    r = np.asarray(in1, np.float64).reshape(P, 1)[:, 0].copy()
    out = np.empty_like(u)
    for t in range(u.shape[1]):
        r = w[:, 0] * r + u[:, t]
        out[:, t] = r
    return out.reshape(in0.shape).astype(np.float32)


class _HandDveOp(DveOp):
    def compile(self, ver):
        if ver != "v3":
            raise ValueError(f"{self.name}: hand-built for v3/TRN2 only")
        cached = dve_ops._COMPILE_CACHE.get((self.name, ver))
        if cached is not None:
            return cached
        spec = DveOpSpec(
            name=self.name,
            opcode=dve_ops.get_dve_sub_opcode(self.name),
            uops=_build_ema_uops(),
            rd1_en=True,
        )
        dve_ops._COMPILE_CACHE[(self.name, ver)] = spec
        return spec


def _register_ema_op() -> DveOp:
    for op in dve_ops.OPS:
        if op.name == "EMA_PAGED_ANT":
            return op
    op = _HandDveOp(
        "EMA_PAGED_ANT",
        Spec(
            body=Bin(AluOp.ADD, Bin(AluOp.MULTIPLY, Src0, C1),
                     Bin(AluOp.MULTIPLY, Src1, C0)),
            reference=_ema_ref,
        ),
        subdim=True,
        uops_sha={},
    )
    dve_ops.OPS.append(op)
    dve_ops.CUSTOM_DVE_SPECS[op.name] = op.spec
    dve_ops._SUB_OPCODE_FOR_NAME[op.name] = (
        dve_ops._CUSTOM_DVE_ROW_BASE + len(dve_ops.OPS) - 1
    )
    assert dve_ops._SUB_OPCODE_FOR_NAME[op.name] < 0x20
    return op


# ---------------------------------------------------------------------------
# Kernel
# ---------------------------------------------------------------------------

B, T, D = 16, 4096, 512
NCORES = 8
BL = B // NCORES   # batches per core
P = 128            # partitions
TCW = 1024         # time chunk per pipeline iteration
ND = D // P        # channel chunks of 128 (4)
NK = TCW // P      # 128-row sub-chunks per time chunk (8)
WTC = 128          # wt table padded to 128 f32 columns (512B rows)

FP32 = mybir.dt.float32
BF16 = mybir.dt.bfloat16


def build_program(bl: int = BL, t: int = T) -> bacc.Bacc:
    ema = _register_ema_op()
    ntc = t // TCW
    nc = bacc.Bacc(
        "TRN2",
        target_bir_lowering=False,
        debug=False,
        enable_asserts=False,
        num_devices=NCORES,
    )
    x = nc.dram_tensor("x", (bl, t, D), BF16, kind="ExternalInput").ap()
    wt = nc.dram_tensor("wt", (P, WTC), FP32, kind="ExternalInput").ap()
    y = nc.dram_tensor("y", (bl, t, D), BF16, kind="ExternalOutput").ap()

    with tile.TileContext(nc) as tc, ExitStack() as ctx:
        const_pool = ctx.enter_context(tc.tile_pool(name="const", bufs=1))
        xn_pool = ctx.enter_context(tc.tile_pool(name="xn", bufs=4))
        pin_pool = ctx.enter_context(tc.tile_pool(name="pin", bufs=4, space="PSUM"))
        pout_pool = ctx.enter_context(tc.tile_pool(name="pout", bufs=2, space="PSUM"))
        s_pool = ctx.enter_context(tc.tile_pool(name="s", bufs=10))
        y_pool = ctx.enter_context(tc.tile_pool(name="y", bufs=3))
        carry_pool = ctx.enter_context(tc.tile_pool(name="carry", bufs=1))

        ident = const_pool.tile([P, P], BF16)
        make_identity(nc, ident[:])

        # Host-precomputed per-channel tables (w | 1/w | 1/a | a), channel
        # d = j*128 + p, on the Scalar HWDGE queue (Sync streams x).
        wt_sb = const_pool.tile([P, WTC], FP32)
        nc.scalar.dma_start(wt_sb[:], wt)
        w_sb = wt_sb[:, 0:ND]
        inv_w = wt_sb[:, ND : 2 * ND]
        inv_a = wt_sb[:, 2 * ND : 3 * ND]
        a_sb = wt_sb[:, 3 * ND : 4 * ND]

        # diag(a) per channel chunk: ident row p scaled by a[p] (bf16).
        diags = []
        for j in range(ND):
            dg = const_pool.tile([P, P], BF16, tag=f"diag{j}")
            nc.scalar.mul(dg[:], ident[:], a_sb[:, j : j + 1])
            diags.append(dg)

        inits = carry_pool.tile([P, bl * ND], FP32)

        # b0's first tile is split 256+768 so the first scan only waits on
        # the first 256KB of x; all other chunks are full 1024-step tiles.
        chunks = [(0, 0, 256), (0, 256, 768), (1, 0, 1024)]
        for tci in range(1, ntc):
            for b in range(bl):
                chunks.append((b, tci * TCW, 1024))
        s_prevs = [[None] * ND for _ in range(bl)]
        for ci, (b, t0, clen) in enumerate(chunks):
                nk = clen // P
                xn = xn_pool.tile([P, NK, D], BF16, tag="xn")
                if clen <= 768:
                    pieces = (0, clen)
                else:
                    pieces = (0, clen // 2, clen)
                for r0, r1 in zip(pieces[:-1], pieces[1:]):
                    nc.sync.dma_start(
                        xn[:, r0 // P : r1 // P, :],
                        x[b, t0 + r0 : t0 + r1, :].rearrange(
                            "(k p) d -> p k d", p=P
                        ),
                    )

                # Transpose to (d-part, t-free) PSUM, then scan straight out
                # of PSUM with the custom EMA op.  The k=0 transpose goes
                # first so the carry-seed mul only waits on the first input
                # piece.
                ss = []
                for j in range(ND):
                    pin = pin_pool.tile([P, TCW], BF16, tag="pin")
                    nc.tensor.transpose(
                        pin[:, 0:P], xn[:, 0, j * P : (j + 1) * P], ident[:]
                    )
                    if t0 == 0:
                        # carry r_{-1} = x_0 / a  =>  s_0 = x_0 exactly
                        nc.scalar.mul(
                            inits[:, b * ND + j : b * ND + j + 1],
                            pin[:, 0:1],
                            inv_a[:, j : j + 1],
                        )
                        carry = inits[:, b * ND + j : b * ND + j + 1]
                    else:
                        prev_s, prev_len = s_prevs[b][j]
                        carry = prev_s[:, prev_len - 1 : prev_len]
                    for k in range(1, nk):
                        nc.tensor.transpose(
                            pin[:, k * P : (k + 1) * P],
                            xn[:, k, j * P : (j + 1) * P],
                            ident[:],
                        )
                    s = s_pool.tile([P, TCW], BF16, tag="s", name=f"s{j}_{b}_{t0}")
                    nc.vector._custom_dve(
                        ema,
                        out=s[:, 0:clen].rearrange("p (s n) -> p s n", n=PAGE),
                        in0=pin[:, 0:clen].rearrange("p (s n) -> p s n", n=PAGE),
                        in1=carry,
                        s0=w_sb[:, j : j + 1],
                        s1=inv_w[:, j : j + 1],
                    )
                    ss.append(s)
                    s_prevs[b][j] = (s, clen)

                # Transpose back with diag(a): yout = (a * r)^T = s^T.
                last = ci == len(chunks) - 1
                yout = y_pool.tile([P, NK, D], BF16, tag="y")
                for m in range(nk // 2):
                    pout = pout_pool.tile([P, 2 * D], FP32, tag="pout")
                    for h in range(2):
                        k = 2 * m + h
                        for j in range(ND):
                            nc.tensor.matmul(
                                pout[:, h * D + j * P : (h * D + (j + 1) * P)],
                                ss[j][:, k * P : (k + 1) * P],
                                diags[j][:],
                            )
                    if last and m % 2 == 1:
                        nc.vector.tensor_copy(yout[:, 2 * m : 2 * m + 2, :], pout[:])
                    else:
                        nc.scalar.copy(yout[:, 2 * m : 2 * m + 2, :], pout[:])
                nhalf = 1 if clen <= 512 else 2
                half = clen // nhalf
                for hh in range(nhalf):
                    # Final tile's stores ride the idle Sync/Scalar HWDGE
                    # queues (fast completion); earlier tiles use GpSimd
                    # SWDGE so the compute-side queues stay free.
                    if last:
                        eng = nc.sync if hh == 0 else nc.scalar
                    else:
                        eng = nc.gpsimd
                    eng.dma_start(
                        y[b, t0 + hh * half : t0 + (hh + 1) * half, :].rearrange(
                            "(k p) d -> p k d", p=P
                        ),
                        yout[:, hh * (half // P) : (hh + 1) * (half // P), :],
                    )

    nc.compile()
    return nc


_prog = None


def make_in_maps(x, alpha):
    x = np.asarray(x)
    alpha = np.asarray(alpha, dtype=np.float64)
    assert x.shape == (B, T, D) and alpha.shape == (1, 1, D)
    xb = np.ascontiguousarray(x.astype(ml_dtypes.bfloat16))
    # Per-channel tables (w | 1/w | 1/a | a) as [128, WTC], channel
    # d = j*128 + p; padded to 512B rows so the DMA runs at line rate.
    a = 1.0 / (1.0 + np.exp(-alpha[0, 0, :]))  # (D,)
    w = 1.0 - a
    wt = np.zeros((P, WTC), np.float32)
    for bi, tb in enumerate((w, 1.0 / w, 1.0 / a, a)):
        wt[:, bi * ND : (bi + 1) * ND] = tb.reshape(ND, P).T
    return [
        {"x": np.ascontiguousarray(xb[i * BL : (i + 1) * BL]), "wt": wt}
        for i in range(NCORES)
    ]


def kernel(x, alpha):
    global _prog
    if _prog is None:
        _prog = build_program()
    in_maps = make_in_maps(x, alpha)
    res = run_bass_kernel_spmd(_prog, in_maps, core_ids=list(range(NCORES)))
    out = np.concatenate([r["y"] for r in res.results], axis=0)
    return np.ascontiguousarray(out.astype(np.float32))


# revision 24
# speedup vs baseline: 1.1554x; 1.0384x over previous
"""Exponential smoothing (per-channel EMA over time) on 8 Trainium2 cores.

  s_0 = x_0 ; s_t = a * x_t + (1 - a) * s_{t-1},  a = sigmoid(alpha)  (per channel)

Full shapes: x (16, 4096, 512) f32, alpha (1, 1, 512) f32 -> out (16, 4096, 512).
Sharding: data-parallel over batch B (16 -> 2 per core); alpha replicated.

Per core, per 1024-step time chunk:
  1. DMA-loads x (cast to bf16 on host) in native layout (t on partitions)
     on the Sync HWDGE queue; the host-precomputed per-channel tables
     (w | 1/w | 1/a | a, padded to 512B rows for line-rate DMA) ride the
     Scalar HWDGE queue in parallel, so no sigmoid/reciprocal/activation-
     table chain gates the first scan.
  2. Transposes 128x128 blocks on the tensor engine into 1-bank PSUM tiles
     (time on the free axis, channels on partitions).  The k=0 transpose is
     emitted first and immediately feeds the carry-seed mul, so the first
     scan only waits on the first 256KB of x.
  3. Runs a hand-built custom DVE op (EMA_PAGED_ANT, registered below) that
     scans r_t = w*r_{t-1} + x_t directly out of PSUM at ~1.1 cyc/element
     (vs ~2.3 for the stock TensorTensorScanArith), writing bf16 r to SBUF.
     The scan is the rescaled form r = s/a, so no pre-scale pass is needed;
     chunk chaining passes the previous chunk's last column as the carry.
     Chunk 0 seeds with r_{-1} = x_0/a, making s_0 = x_0 exactly.
  4. Transposes back via a regular matmul against diag(a) (contracting the
     channel partitions), so s = a*r is applied by the tensor engine for free.
  5. Scalar engine evacuates PSUM -> bf16 SBUF; stores go out on the GpSimd
     SWDGE queue, except the final tile's, which use the (by then idle)
     Sync/Scalar HWDGE queues and the vector engine for the last evacs so
     the tail drains fast.
Host converts the bf16 y back to f32.
"""

from contextlib import ExitStack

import ml_dtypes
import numpy as np

import concourse.bass as bass
import concourse.dve_ops as dve_ops
import concourse.tile as tile
from concourse import bacc, mybir
from concourse.bass_utils import run_bass_kernel_spmd
from concourse.dve_ops import DveOp
from concourse.dve_spec import (
    C0,
    C1,
    AluOp,
    Bin,
    One,
    Spec,
    Src0,
    Src1,
    _Placement,
    _Stage,
    _State,
    _assemble,
    COUNT_ONCE,
    PREV,
)
from concourse.dve_uop import AluInp, DveOpSpec, OutSel, Trigger
from concourse.masks import make_identity

# ---------------------------------------------------------------------------
# Custom DVE op: paged EMA scan, r_k = w*r_{k-1} + u_k at ~1.1 cyc/element.
# Within each 16-element page the weights are formed as w^(i+1) * w^-(j+1)
# via three no-bubble running products/sums; two non-consuming bubble uOps at
# each page boundary rescale the carry by w^16 exactly. fp32 range needs
# w^-15 finite: OK for w >= sigmoid(-5.8).
# ---------------------------------------------------------------------------

CURR = AluInp.CURR_ALU_OUT
SWAP = AluInp.CURR_SWAP_OUT
LANE_M = AluInp.PREV_DELAY_3
LANE_R = AluInp.PREV_DELAY_4
PAGE = 16
_BYP = _Stage(AluOp.BYPASS, PREV)


def _build_ema_uops():
    m_key = Bin(AluOp.MULTIPLY, Src0, C1)
    r_key = Bin(AluOp.MULTIPLY, C0, C0)
    p = _Placement(
        pipeline=[
            _Stage(AluOp.MULTIPLY, CURR, C1),     # st0: Inv <- Inv * (1/w)
            _Stage(AluOp.MULTIPLY, Src0, PREV),   # st1: m = u * Inv
            _Stage(AluOp.MULTIPLY, CURR, C0),     # st2: R <- R * w
            _Stage(AluOp.ADD, CURR, LANE_M),      # st3: A <- A + m
            _Stage(AluOp.MULTIPLY, PREV, LANE_R), # st4: out = A * R
            _BYP, _BYP, _BYP,
        ],
        node_stage={},
        lane={Src0: 0, C0: 1, C1: 2, m_key: 3, r_key: 4, One: 5},
        out_sel=OutSel.ALU_OUT,
        accum_stage=None,
        captures=[(2, 3), (3, 4)],
    )
    latch_p = _Placement(
        pipeline=[_BYP] * 8, node_stage={}, lane={Src1: 0},
        out_sel=OutSel.ALU_OUT, accum_stage=None, captures=[],
    )
    states = [
        _State(  # 0: latch-init — park the carry (in1) in st3's swap flop
            placement=latch_p, trigger=COUNT_ONCE, repeat=1,
            consume=(False, True),
            overrides={3: _Stage(AluOp.BYPASS, Src1, Src1, swap=True)},
            write_out=False, next=(1, 0, 0),
        ),
        _State(  # 1: seed — Inv=1, R=1, A=carry
            placement=p, trigger=COUNT_ONCE, repeat=1, consume=(False, False),
            overrides={
                0: _Stage(AluOp.BYPASS, One),
                2: _Stage(AluOp.BYPASS, One),
                3: _Stage(AluOp.BYPASS, SWAP),
            },
            write_out=False, next=(2, 0, 0),
        ),
        _State(  # 2: steady — 1 element/cycle; page wrap -> bubbles
            placement=p,
            trigger=(Trigger.SRC_TENSOR_DONE, Trigger.SUB_DIM_DONE, Trigger.NONE),
            next=(0, 3, 0), repeat=0, consume=(True, False), write_out=True,
        ),
        _State(  # 3: B1 — A <- A * w^PAGE (R held at st2)
            placement=p, trigger=COUNT_ONCE, repeat=1, consume=(False, False),
            overrides={
                0: _Stage(AluOp.BYPASS, CURR), 1: _BYP,
                2: _Stage(AluOp.BYPASS, CURR),
                3: _Stage(AluOp.MULTIPLY, CURR, PREV), 4: _BYP,
            },
            write_out=False, next=(4, 0, 0),
        ),
        _State(  # 4: B2 — reset Inv/R for the new page, hold A
            placement=p, trigger=COUNT_ONCE, repeat=1, consume=(False, False),
            overrides={
                0: _Stage(AluOp.BYPASS, One), 1: _BYP,
                2: _Stage(AluOp.BYPASS, One),
                3: _Stage(AluOp.BYPASS, CURR), 4: _BYP,
            },
            write_out=False, next=(2, 0, 0),
        ),
    ]
    uops = [_assemble(s) for s in states]
    for u in uops:
        u.validate("v3")
    return uops


def _ema_ref(in0, in1, s0, s1, imm2):
    P = in0.shape[0]
    u = in0.astype(np.float64).reshape(P, -1)
    w = np.asarray(s0, np.float64).reshape(P, 1)
    r = np.asarray(in1, np.float64).reshape(P, 1)[:, 0].copy()
    out = np.empty_like(u)
    for t in range(u.shape[1]):
        r = w[:, 0] * r + u[:, t]
        out[:, t] = r
    return out.reshape(in0.shape).astype(np.float32)


class _HandDveOp(DveOp):
    def compile(self, ver):
        if ver != "v3":
            raise ValueError(f"{self.name}: hand-built for v3/TRN2 only")
        cached = dve_ops._COMPILE_CACHE.get((self.name, ver))
        if cached is not None:
            return cached
        spec = DveOpSpec(
            name=self.name,
            opcode=dve_ops.get_dve_sub_opcode(self.name),
            uops=_build_ema_uops(),
            rd1_en=True,
        )
        dve_ops._COMPILE_CACHE[(self.name, ver)] = spec
        return spec


def _register_ema_op() -> DveOp:
    for op in dve_ops.OPS:
        if op.name == "EMA_PAGED_ANT":
            return op
    op = _HandDveOp(
        "EMA_PAGED_ANT",
        Spec(
            body=Bin(AluOp.ADD, Bin(AluOp.MULTIPLY, Src0, C1),
                     Bin(AluOp.MULTIPLY, Src1, C0)),
            reference=_ema_ref,
        ),
        subdim=True,
        uops_sha={},
    )
    dve_ops.OPS.append(op)
    dve_ops.CUSTOM_DVE_SPECS[op.name] = op.spec
    dve_ops._SUB_OPCODE_FOR_NAME[op.name] = (
        dve_ops._CUSTOM_DVE_ROW_BASE + len(dve_ops.OPS) - 1
    )
    assert dve_ops._SUB_OPCODE_FOR_NAME[op.name] < 0x20
    return op


# ---------------------------------------------------------------------------
# Kernel
# ---------------------------------------------------------------------------

B, T, D = 16, 4096, 512
NCORES = 8
BL = B // NCORES   # batches per core
P = 128            # partitions
TCW = 1024         # time chunk per pipeline iteration
ND = D // P        # channel chunks of 128 (4)
NK = TCW // P      # 128-row sub-chunks per time chunk (8)
WTC = 128          # wt table padded to 128 f32 columns (512B rows)

FP32 = mybir.dt.float32
BF16 = mybir.dt.bfloat16


def build_program(bl: int = BL, t: int = T) -> bacc.Bacc:
    ema = _register_ema_op()
    ntc = t // TCW
    nc = bacc.Bacc(
        "TRN2",
        target_bir_lowering=False,
        debug=False,
        enable_asserts=False,
        num_devices=NCORES,
    )
    x = nc.dram_tensor("x", (bl, t, D), BF16, kind="ExternalInput").ap()
    wt = nc.dram_tensor("wt", (P, WTC), FP32, kind="ExternalInput").ap()
    y = nc.dram_tensor("y", (bl, t, D), BF16, kind="ExternalOutput").ap()

    with tile.TileContext(nc) as tc, ExitStack() as ctx:
        const_pool = ctx.enter_context(tc.tile_pool(name="const", bufs=1))
        xn_pool = ctx.enter_context(tc.tile_pool(name="xn", bufs=4))
        pin_pool = ctx.enter_context(tc.tile_pool(name="pin", bufs=4, space="PSUM"))
        pout_pool = ctx.enter_context(tc.tile_pool(name="pout", bufs=2, space="PSUM"))
        s_pool = ctx.enter_context(tc.tile_pool(name="s", bufs=10))
        y_pool = ctx.enter_context(tc.tile_pool(name="y", bufs=3))
        carry_pool = ctx.enter_context(tc.tile_pool(name="carry", bufs=1))

        ident = const_pool.tile([P, P], BF16)
        make_identity(nc, ident[:])

        # Host-precomputed per-channel tables (w | 1/w | 1/a | a), channel
        # d = j*128 + p, on the Scalar HWDGE queue (Sync streams x).
        wt_sb = const_pool.tile([P, WTC], FP32)
        nc.scalar.dma_start(wt_sb[:], wt)
        w_sb = wt_sb[:, 0:ND]
        inv_w = wt_sb[:, ND : 2 * ND]
        inv_a = wt_sb[:, 2 * ND : 3 * ND]
        a_sb = wt_sb[:, 3 * ND : 4 * ND]

        diags = []  # built lazily after the first chunk's scans are queued,
        # so the scalar engine runs the carry-seed muls first

        inits = carry_pool.tile([P, bl * ND], FP32)

        # b0's first tile is split 256+768 so the first scan only waits on
        # the first 256KB of x; all other chunks are full 1024-step tiles.
        chunks = [(0, 0, 256), (0, 256, 768), (1, 0, 1024)]
        for tci in range(1, ntc):
            for b in range(bl):
                chunks.append((b, tci * TCW, 1024))
        s_prevs = [[None] * ND for _ in range(bl)]
        for ci, (b, t0, clen) in enumerate(chunks):
                nk = clen // P
                xn = xn_pool.tile([P, NK, D], BF16, tag="xn")
                if clen <= 768:
                    pieces = (0, clen)
                else:
                    pieces = (0, clen // 2, clen)
                for r0, r1 in zip(pieces[:-1], pieces[1:]):
                    nc.sync.dma_start(
                        xn[:, r0 // P : r1 // P, :],
                        x[b, t0 + r0 : t0 + r1, :].rearrange(
                            "(k p) d -> p k d", p=P
                        ),
                    )

                # Transpose to (d-part, t-free) PSUM, then scan straight out
                # of PSUM with the custom EMA op.  The k=0 transpose goes
                # first so the carry-seed mul only waits on the first input
                # piece.
                last = ci == len(chunks) - 1
                ss = []
                scan_args = []
                for j in range(ND):
                    pin = pin_pool.tile([P, TCW], BF16, tag="pin")
                    nc.tensor.transpose(
                        pin[:, 0:P], xn[:, 0, j * P : (j + 1) * P], ident[:]
                    )
                    if t0 == 0:
                        # carry r_{-1} = x_0 / a  =>  s_0 = x_0 exactly
                        nc.scalar.mul(
                            inits[:, b * ND + j : b * ND + j + 1],
                            pin[:, 0:1],
                            inv_a[:, j : j + 1],
                        )
                        carry = inits[:, b * ND + j : b * ND + j + 1]
                    else:
                        prev_s, prev_len = s_prevs[b][j]
                        carry = prev_s[:, prev_len - 1 : prev_len]
                    for k in range(1, nk):
                        nc.tensor.transpose(
                            pin[:, k * P : (k + 1) * P],
                            xn[:, k, j * P : (j + 1) * P],
                            ident[:],
                        )
                    s = s_pool.tile([P, TCW], BF16, tag="s", name=f"s{j}_{b}_{t0}")

                    def emit_scan(j, pin, carry, s, c0, c1):
                        cr = carry if c0 == 0 else s[:, c0 - 1 : c0]
                        nc.vector._custom_dve(
                            ema,
                            out=s[:, c0:c1].rearrange("p (s n) -> p s n", n=PAGE),
                            in0=pin[:, c0:c1].rearrange("p (s n) -> p s n", n=PAGE),
                            in1=cr,
                            s0=w_sb[:, j : j + 1],
                            s1=inv_w[:, j : j + 1],
                        )

                    if not last:
                        emit_scan(j, pin, carry, s, 0, clen)
                    ss.append(s)
                    scan_args.append((pin, carry, s))
                    s_prevs[b][j] = (s, clen)
                if last:
                    # Halved scans so the back-pass (and the PE clock-gate
                    # warm-up it depends on) overlaps the final scans.
                    for j, (pin, carry, s) in enumerate(scan_args):
                        emit_scan(j, pin, carry, s, 0, clen // 2)

                # Transpose back with diag(a): yout = (a * r)^T = s^T.
                if not diags:
                    # Built here (not at program start) so the scalar engine
                    # runs the first carry-seed muls before these.
                    for j in range(ND):
                        dg = const_pool.tile([P, P], BF16, tag=f"diag{j}")
                        nc.scalar.mul(dg[:], ident[:], a_sb[:, j : j + 1])
                        diags.append(dg)

                yout = y_pool.tile([P, NK, D], BF16, tag="y")

                def do_pout(m, vec_evac=False):
                    pout = pout_pool.tile([P, 2 * D], FP32, tag="pout")
                    for h in range(2):
                        k = 2 * m + h
                        for j in range(ND):
                            nc.tensor.matmul(
                                pout[:, h * D + j * P : (h * D + (j + 1) * P)],
                                ss[j][:, k * P : (k + 1) * P],
                                diags[j][:],
                            )
                    if vec_evac:
                        nc.vector.tensor_copy(yout[:, 2 * m : 2 * m + 2, :], pout[:])
                    else:
                        nc.scalar.copy(yout[:, 2 * m : 2 * m + 2, :], pout[:])

                def out_piece(eng, r0, r1):
                    eng.dma_start(
                        y[b, t0 + r0 : t0 + r1, :].rearrange(
                            "(k p) d -> p k d", p=P
                        ),
                        yout[:, r0 // P : r1 // P, :],
                    )

                if not last:
                    for m in range(nk // 2):
                        do_pout(m)
                    if clen <= 512:
                        out_piece(nc.gpsimd, 0, clen)
                    else:
                        out_piece(nc.gpsimd, 0, clen // 2)
                        out_piece(nc.gpsimd, clen // 2, clen)
                else:
                    # Back-pass chases the halved scans; the final stores
                    # split across the idle Sync/Scalar HWDGE queues and the
                    # last evac runs on the (idle by then) vector engine.
                    do_pout(0)
                    do_pout(1)
                    out_piece(nc.sync, 0, clen // 2)
                    for j, (pin, carry, s) in enumerate(scan_args):
                        emit_scan(j, pin, carry, s, clen // 2, clen)
                    do_pout(2)
                    do_pout(3, vec_evac=True)
                    out_piece(nc.scalar, clen // 2, 3 * clen // 4)
                    out_piece(nc.sync, 3 * clen // 4, clen)

    nc.compile()
    return nc


_prog = None


def make_in_maps(x, alpha):
    x = np.asarray(x)
    alpha = np.asarray(alpha, dtype=np.float64)
    assert x.shape == (B, T, D) and alpha.shape == (1, 1, D)
    xb = np.ascontiguousarray(x.astype(ml_dtypes.bfloat16))
    # Per-channel tables (w | 1/w | 1/a | a) as [128, WTC], channel
    # d = j*128 + p; padded to 512B rows so the DMA runs at line rate.
    a = 1.0 / (1.0 + np.exp(-alpha[0, 0, :]))  # (D,)
    w = 1.0 - a
    wt = np.zeros((P, WTC), np.float32)
    for bi, tb in enumerate((w, 1.0 / w, 1.0 / a, a)):
        wt[:, bi * ND : (bi + 1) * ND] = tb.reshape(ND, P).T
    return [
        {"x": np.ascontiguousarray(xb[i * BL : (i + 1) * BL]), "wt": wt}
        for i in range(NCORES)
    ]


def kernel(x, alpha):
    global _prog
    if _prog is None:
        _prog = build_program()
    in_maps = make_in_maps(x, alpha)
    res = run_bass_kernel_spmd(_prog, in_maps, core_ids=list(range(NCORES)))
    out = np.concatenate([r["y"] for r in res.results], axis=0)
    return np.ascontiguousarray(out.astype(np.float32))
